# revision 10
# baseline (speedup 1.0000x reference)
"""Trainium2 Bass kernel for nn_BilinearEquivariantLayer.

Pipeline (per core c of 8, SPMD):
  stage 1: A_pos[k] = P[k] @ V[k] for k in {2c, 2c+1}      (k-sharded)
  AllToAll: redistribute A_pos so core c holds all k for its 64-col r-block
  stage 2: irfft over k as a (32->31) matmul (CIR)
  stage 3: W1A/W2A = W @ A_real  (own r-block; bf16 out)
  AllGather: W1A across cores -> full R (overlapped with W2A pass)
  stage 4: bilinear U[t,h] = W2A[t,h].T @ W1A[t,h]  (own s-block, full r)
  stage 5: fused rfft+mixer as one (248->256) matmul (G), direct to output
"""
import sys
sys.path.insert(0, "/opt/trn_rl_repo")
import os
import numpy as np
from concourse import bass, bacc, tile, mybir
from concourse import bass_utils

NCORES = 8
K, D, N, R, H, dproj = 16, 512, 1024, 512, 8, 128
T = 2 * K - 1           # 31
KL = K // NCORES        # 2 k's per core
RC = R // NCORES        # 64 r-cols per core
F32 = mybir.dt.float32
F32R = mybir.dt.float32r
BF16 = mybir.dt.bfloat16

_CACHE = {}


def _build():
    nc = bacc.Bacc("TRN2", target_bir_lowering=False, debug=False,
                   num_devices=NCORES)
    pt = nc.dram_tensor("pt", [KL, 2, N, D], F32R, kind="ExternalInput").ap()
    v = nc.dram_tensor("v", [KL, N, R], F32R, kind="ExternalInput").ap()
    w1t = nc.dram_tensor("w1t", [D, H * dproj], F32R, kind="ExternalInput").ap()
    w2t = nc.dram_tensor("w2t", [D, H * dproj], F32R, kind="ExternalInput").ap()
    cir = nc.dram_tensor("cir", [2 * K, T], F32R, kind="ExternalInput").ap()
    g = nc.dram_tensor("g", [2, 124, 256], F32R, kind="ExternalInput").ap()
    o = nc.dram_tensor("o", [2, K, H, RC, R], F32, kind="ExternalOutput").ap()

    with tile.TileContext(nc) as tc:
        with tc.tile_pool(name="dram", bufs=1, space="DRAM") as dram:
            a2a_in = dram.tile([NCORES, KL, 2, D, RC], F32R)
            a2a_out = dram.tile([NCORES, KL, 2, D, RC], F32R)
            a_real = dram.tile([4, 128, 4, T, 16], F32R)
            w1a_loc = dram.tile([H, dproj, T, RC], BF16)
            w2a_loc = dram.tile([H, dproj, T, RC], BF16)
            w1a_ag = dram.tile([NCORES, H, dproj, T, RC], BF16,
                               addr_space="Shared")
            u_dram = dram.tile([2, 124, RC, R], F32R)

            # ---- stage 1: A_pos = P @ V (own k's) -> a2a_in
            with tc.tile_pool(name="s1", bufs=1) as s1, \
                 tc.tile_pool(name="s1c", bufs=3) as s1c, \
                 tc.tile_pool(name="ps1", bufs=3, space="PSUM") as ps1p:
                pt_sb = s1.tile([128, KL, 2, 8, D], F32R)
                v_sb = s1.tile([128, KL, 8, R], F32R)
                for kl in range(KL):
                    for nci in range(8):
                        nc.sync.dma_start(
                            out=v_sb[:, kl, nci, :],
                            in_=v[kl, nci * 128:(nci + 1) * 128, :])
                        for ri in range(2):
                            nc.sync.dma_start(
                                out=pt_sb[:, kl, ri, nci, :],
                                in_=pt[kl, ri, nci * 128:(nci + 1) * 128, :])
                for kl in range(KL):
                    for ri in range(2):
                        for dc in range(4):
                            ps1 = ps1p.tile([128, R], F32, tag="ps1")
                            for nci in range(8):
                                nc.tensor.matmul(
                                    ps1[:],
                                    pt_sb[:, kl, ri, nci,
                                          dc * 128:(dc + 1) * 128],
                                    v_sb[:, kl, nci, :],
                                    start=(nci == 0), stop=(nci == 7))
                            cp1 = s1c.tile([128, R], F32R, tag="cp1")
                            nc.vector.tensor_copy(cp1[:], ps1[:])
                            nc.sync.dma_start(
                                out=a2a_in[:, kl, ri,
                                           dc * 128:(dc + 1) * 128,
                                           :].transpose([1, 0, 2]),
                                in_=cp1[:].rearrange("p (rb rc) -> p rb rc",
                                                     rb=8))

            nc.gpsimd.collective_compute(
                "AllToAll", mybir.AluOpType.bypass,
                replica_groups=[list(range(NCORES))],
                ins=[a2a_in.opt()], outs=[a2a_out.opt()])

            # ---- stage 2: irfft (32 -> 31 over k) -> a_real [dc,dl,t,rc]
            a2a_flat = a2a_out[:].rearrange("s kl ri dd rc -> (s kl ri) (dd rc)")
            with tc.tile_pool(name="s2", bufs=1) as s2, \
                 tc.tile_pool(name="s2r", bufs=4) as s2r, \
                 tc.tile_pool(name="ps2", bufs=4, space="PSUM") as ps2p:
                cir_sb = s2.tile([2 * K, T], F32R)
                nc.sync.dma_start(out=cir_sb[:], in_=cir[:, :])
                for ci in range(64):
                    rhs2 = s2r.tile([2 * K, 512], F32R, tag="rhs2")
                    nc.sync.dma_start(
                        out=rhs2[:], in_=a2a_flat[:, ci * 512:(ci + 1) * 512])
                    ps2 = ps2p.tile([T, 512], F32, tag="ps2")
                    nc.tensor.matmul(ps2[:], cir_sb[:],
                                     rhs2[:],
                                     start=True, stop=True)
                    dc, dl0 = ci // 16, (ci * 8) % 128
                    cp2 = s2r.tile([T, 512], F32R, tag="cp2")
                    nc.vector.tensor_copy(cp2[:], ps2[:])
                    nc.sync.dma_start(
                        out=a_real[dc, dl0:dl0 + 8, :, :,
                                   :].transpose([2, 0, 1, 3]),
                        in_=cp2[:].rearrange("t (j rcb rc) -> t j rcb rc",
                                             j=8, rcb=4))

            # ---- stage 3: W1A / W2A (own r-block), bf16; AG(W1A) overlaps W2A
            with tc.tile_pool(name="s3", bufs=1) as s3, \
                 tc.tile_pool(name="s3c", bufs=4) as s3c, \
                 tc.tile_pool(name="ps3", bufs=4, space="PSUM") as ps3p:
                ar_sb = s3.tile([128, 4, 4, T, 16], F32R)
                for dc in range(4):
                    nc.sync.dma_start(
                        out=ar_sb[:, dc, :, :, :], in_=a_real[dc, :, :, :, :])
                w1t_sb = s3.tile([128, 4, H * dproj], F32R)
                w2t_sb = s3.tile([128, 4, H * dproj], F32R)
                for dc in range(4):
                    nc.sync.dma_start(
                        out=w1t_sb[:, dc, :],
                        in_=w1t[dc * 128:(dc + 1) * 128, :])
                    nc.sync.dma_start(
                        out=w2t_sb[:, dc, :],
                        in_=w2t[dc * 128:(dc + 1) * 128, :])
                for w_sb, w_loc in ((w1t_sb, w1a_loc), (w2t_sb, w2a_loc)):
                    for h in range(H):
                        for rcb in range(4):
                            ps3 = ps3p.tile([128, T * 16], F32, tag="ps3")
                            for dc in range(4):
                                nc.tensor.matmul(
                                    ps3[:],
                                    w_sb[:, dc, h * 128:(h + 1) * 128
                                         ],
                                    ar_sb[:, dc, rcb].rearrange(
                                        "p t rc -> p (t rc)"),
                                    start=(dc == 0), stop=(dc == 3))
                            wcast = s3c.tile([128, T * 16], BF16, tag="wcast")
                            nc.vector.tensor_copy(wcast[:], ps3[:])
                            nc.sync.dma_start(
                                out=w_loc[h, :, :, rcb * 16:(rcb + 1) * 16],
                                in_=wcast[:].rearrange("p (t rc) -> p t rc",
                                                       t=T))
                    if w_loc is w1a_loc:
                        nc.gpsimd.collective_compute(
                            "AllGather", mybir.AluOpType.bypass,
                            replica_groups=[list(range(NCORES))],
                            ins=[w1a_loc.opt()], outs=[w1a_ag.opt()])

            # ---- stage 4: bilinear -> u_dram ; stage 5: G matmul -> o
            with tc.tile_pool(name="s4", bufs=1) as s4, \
                 tc.tile_pool(name="s4w", bufs=2) as s4w, \
                 tc.tile_pool(name="ps4", bufs=4, space="PSUM") as ps4p:
                w2a_sb = s4.tile([dproj, H, T, RC], BF16)
                for h in range(H):
                    nc.sync.dma_start(
                        out=w2a_sb[:, h, :, :], in_=w2a_loc[h, :, :, :])
                for h in range(H):
                    w1a_sb = s4w.tile([dproj, T, R], BF16,
                                      tag="w1a_sb")
                    for rb in range(NCORES):
                        nc.sync.dma_start(
                            out=w1a_sb[:, :, rb * RC:(rb + 1) * RC],
                            in_=w1a_ag[rb, h, :, :, :])
                    for t in range(T):
                        ps4 = ps4p.tile([64, R], F32, tag="ps4")
                        nc.tensor.matmul(
                            ps4[:], w2a_sb[:, h, t, :],
                            w1a_sb[:, t, :],
                            start=True, stop=True)
                        cp4 = s4w.tile([64, R], F32R, tag="cp4", bufs=4)
                        nc.vector.tensor_copy(cp4[:], ps4[:])
                        nc.sync.dma_start(
                            out=u_dram[h // 4, (h % 4) * T + t, :, :],
                            in_=cp4[:])

            u_flat = u_dram[:].rearrange("c p sc r -> c p (sc r)")
            with tc.tile_pool(name="s5", bufs=1) as s5, \
                 tc.tile_pool(name="s5r", bufs=4) as s5r, \
                 tc.tile_pool(name="ps5", bufs=4, space="PSUM") as ps5p:
                g_sb = s5.tile([124, 2, 256], F32R)
                for cc in range(2):
                    nc.sync.dma_start(out=g_sb[:, cc, :], in_=g[cc, :, :])
                o_flat = o.rearrange("m k j sc r -> m (k j) (sc r)")
                for mc in range(2):
                    for fc in range(64):
                        ps5 = ps5p.tile([128, 512], F32, tag="ps5")
                        for cc in range(2):
                            urhs = s5r.tile([124, 512], F32R, tag="urhs")
                            nc.sync.dma_start(
                                out=urhs[:],
                                in_=u_flat[cc, :, fc * 512:(fc + 1) * 512])
                            nc.tensor.matmul(
                                ps5[:],
                                g_sb[:, cc, mc * 128:(mc + 1) * 128
                                     ],
                                urhs[:],
                                start=(cc == 0), stop=(cc == 1))
                        cp5 = s5r.tile([128, 512], F32, tag="cp5")
                        nc.vector.tensor_copy(cp5[:], ps5[:])
                        nc.sync.dma_start(
                            out=o_flat[mc, :, fc * 512:(fc + 1) * 512],
                            in_=cp5[:])

    nc.compile()
    return nc


def _host_prep(P_real, P_imag, V, W1, W2, mixer_real, mixer_imag):
    P_real = np.asarray(P_real, np.float32)
    P_imag = np.asarray(P_imag, np.float32)
    V = np.asarray(V, np.float32)
    W1 = np.asarray(W1, np.float32)
    W2 = np.asarray(W2, np.float32)
    mr = np.asarray(mixer_real, np.float32)
    mi = np.asarray(mixer_imag, np.float32)

    pt_all = np.stack([P_real.transpose(0, 2, 1),
                       P_imag.transpose(0, 2, 1)], axis=1)  # (K, 2, N, D)
    w1t = np.ascontiguousarray(W1.reshape(H * dproj, D).T)
    w2t = np.ascontiguousarray(W2.reshape(H * dproj, D).T)

    t_idx, k_idx = np.arange(T), np.arange(K)
    ang = 2 * np.pi * np.outer(k_idx, t_idx) / T
    scale = np.where(k_idx[:, None] == 0, 1.0, 2.0) / T
    cir = np.empty((2 * K, T), np.float32)
    cir[0::2] = np.cos(ang) * scale
    cir[1::2] = -np.sin(ang) * scale

    cos2, sin2 = np.cos(ang), np.sin(ang)  # (K, T)
    G = np.empty((H, T, 2, K, H), np.float32)
    G[:, :, 0] = (np.einsum('kt,ij->itkj', cos2, mr)
                  + np.einsum('kt,ij->itkj', sin2, mi))
    G[:, :, 1] = (np.einsum('kt,ij->itkj', cos2, mi)
                  - np.einsum('kt,ij->itkj', sin2, mr))
    g = np.ascontiguousarray(G.reshape(2, 124, 256))

    in_maps = []
    for c in range(NCORES):
        in_maps.append({
            "pt": np.ascontiguousarray(pt_all[2 * c:2 * c + 2]),
            "v": np.ascontiguousarray(V[2 * c:2 * c + 2]),
            "w1t": w1t, "w2t": w2t, "cir": cir, "g": g,
        })
    return in_maps


def _assemble(outs):
    res = np.empty((K, R, R, H), np.complex64)
    for c in range(NCORES):
        oc = outs[c]  # (2, K, H, RC, R)
        res[:, :, c * RC:(c + 1) * RC, :] = (
            oc[0] + 1j * oc[1]).transpose(0, 3, 2, 1)
    return res


def _enable_axon_trace():
    """Dev-only: register the NTFF profile hook (missing antenv.axon_hooks)
    and stub the artifact upload so run_bass_kernel_spmd(trace=True) works."""
    import types
    if "antenv.axon_hooks" not in sys.modules:
        m = types.ModuleType("antenv.axon_hooks")
        m._hook = None
        m.set_axon_ntff_profile_hook = lambda h: setattr(m, "_hook", h)
        m.get_axon_ntff_profile_hook = lambda: m._hook
        sys.modules["antenv.axon_hooks"] = m
        import antenv
        antenv.axon_hooks = m
        from trn_agent_boot.trn_boot import _ntff_profile_via_ctypes
        hook = _ntff_profile_via_ctypes("/opt/axon/libaxon_pjrt.so")
        m._hook = hook
    bass_utils.upload_artifacts = lambda tmpdir: f"local:{tmpdir}"


def kernel(P_real, P_imag, V, W1, W2, mixer_real, mixer_imag):
    if "nc" not in _CACHE:
        _CACHE["nc"] = _build()
    nc = _CACHE["nc"]
    in_maps = _host_prep(P_real, P_imag, V, W1, W2, mixer_real, mixer_imag)

    if os.environ.get("KSIM"):
        from concourse.bass_interp import MultiCoreSim
        sim = MultiCoreSim(nc, num_cores=NCORES, num_workers=NCORES)
        for c in range(NCORES):
            for k_, arr in in_maps[c].items():
                sim.cores[c].tensor(k_)[:] = arr
        sim.simulate(check_with_hw=False)
        outs = [np.array(sim.cores[c].tensor("o")) for c in range(NCORES)]
        return _assemble(outs)

    trace = bool(os.environ.get("KTRACE"))
    if trace:
        _enable_axon_trace()
    res = bass_utils.run_bass_kernel_spmd(
        nc, in_maps, core_ids=list(range(NCORES)), trace=trace,
        tmpdir=os.environ.get("KTRACE_DIR") or None)
    if trace:
        print(f"HW exec time: {res.exec_time_ns} ns")
        _CACHE["exec_time_ns"] = res.exec_time_ns
        _CACHE["results"] = res
    outs = [res.results[c]["o"] for c in range(NCORES)]
    return _assemble(outs)
